# revision 28
# baseline (speedup 1.0000x reference)
"""Trainium2 Bass kernel for the Graves GMM attention-window mechanism.

Computes, for each batch row b:
    raw   = x @ W + b                    (folded: ones-column + W_aug row)
    alpha = exp(raw[:, 0:10]) ; beta = exp(raw[:, 10:20])
    kappa = prev_kappa + exp(raw[:, 20:30] + kappa_scale)
    phi[l] = sum_k alpha_k * exp(clip(-beta_k (kappa_k - u_l)^2, -50, 50))
    phi   *= (l < seqlen)
    w[c]  = sum_l phi[l] * char_seq[l, c]

Sharding: pure data parallel over 8 NeuronCores, 512 rows each. Rows are
sorted by sequence length on the host so that fully-masked char chunks can
be skipped statically (the chunk counts are baked into the compiled graph,
derived from the actual lengths of this call's inputs).
"""

from contextlib import ExitStack

import numpy as np

import concourse.bacc as bacc
import concourse.bass as bass
import concourse.tile as tile
from concourse import mybir
from concourse.bass_utils import run_bass_kernel_spmd

FP32 = mybir.dt.float32
AF = mybir.ActivationFunctionType
ALU = mybir.AluOpType

N_CORES = 8
P = 128          # SBUF partitions (rows per tile)
C = 80           # char vocab
K = 10           # mixture components
L = 256          # max char positions
LC = 32          # l-chunk size for the streamed einsum
LN50 = float(np.log(50.0))

# tuning knobs (swept offline with TimelineSim)
GP_MOD = 7       # chunks with gci % GP_MOD in GP_RES go to GpSimd
GP_RES = (0, 4)
CH_BUFS = 8
PHIM_BUFS = 4
TERMS_BUFS = 2
TREE_BF16 = True  # run the l-reduction tree in packed bf16 (DVE 2x mode)

_cache: dict = {}
last_run = {}    # test.py introspection: exec_time_ns etc.


def _build(n_chunks, d_aug):
    """Build the SPMD graph for one core; n_chunks[rt] = char chunks to
    process for row-tile rt (same for every core)."""
    rt_count = len(n_chunks)
    rows = rt_count * P
    nc = bacc.Bacc("TRN2", target_bir_lowering=False, debug=False,
                   num_devices=N_CORES)

    x_d = nc.dram_tensor("x", [rows, d_aug], FP32, kind="ExternalInput").ap()
    pk_d = nc.dram_tensor("pk", [rows, K], FP32, kind="ExternalInput").ap()
    ch_d = nc.dram_tensor("ch", [rows, L, C], FP32, kind="ExternalInput").ap()
    sl_d = nc.dram_tensor("sl", [rows, 1], FP32, kind="ExternalInput").ap()
    w_d = nc.dram_tensor("wmat", [d_aug, 3 * K], FP32, kind="ExternalInput").ap()
    u_d = nc.dram_tensor("u", [P, L], FP32, kind="ExternalInput").ap()
    id_d = nc.dram_tensor("ident", [P, P], FP32, kind="ExternalInput").ap()
    wout_d = nc.dram_tensor("w_out", [rows, C], FP32, kind="ExternalOutput").ap()
    kout_d = nc.dram_tensor("k_out", [rows, K], FP32, kind="ExternalOutput").ap()

    # contraction chunks over the augmented input dim (401 = 3*128 + 17)
    kchunks = []
    k0 = 0
    while k0 < d_aug:
        kn = min(P, d_aug - k0)
        kchunks.append((k0, kn))
        k0 += kn

    with tile.TileContext(nc) as tc, ExitStack() as ctx:
        const = ctx.enter_context(tc.tile_pool(name="const", bufs=1))
        xpool = ctx.enter_context(tc.tile_pool(name="x", bufs=2))
        ppool = ctx.enter_context(tc.tile_pool(name="phi", bufs=TERMS_BUFS))
        mpool = ctx.enter_context(tc.tile_pool(name="phim", bufs=PHIM_BUFS))
        small = ctx.enter_context(tc.tile_pool(name="small", bufs=4))
        chpool = ctx.enter_context(tc.tile_pool(name="ch", bufs=CH_BUFS))
        opool = ctx.enter_context(tc.tile_pool(name="out", bufs=4))
        psum_t = ctx.enter_context(tc.tile_pool(name="psum_t", bufs=2, space="PSUM"))
        psum_r = ctx.enter_context(tc.tile_pool(name="psum_r", bufs=2, space="PSUM"))

        u_t = const.tile([P, L], FP32, tag="u")
        nc.sync.dma_start(u_t[:], u_d[:])
        ln50_t = const.tile([P, 1], FP32, tag="ln50")
        nc.vector.memset(ln50_t[:], LN50)
        # warm the GpSimd ucode dispatch table before its first real chunk
        gp_warm = const.tile([P, 8], FP32, tag="gpwarm")
        nc.gpsimd.memset(gp_warm[:], 0.0)
        nc.gpsimd.tensor_add(gp_warm[:, 0:4], gp_warm[:, 0:4], gp_warm[:, 4:8])
        id_t = const.tile([P, P], FP32, tag="ident")
        nc.sync.dma_start(id_t[:], id_d[:])
        w_tiles = []
        for i, (k0, kn) in enumerate(kchunks):
            wt = const.tile([kn, 3 * K], FP32, tag=f"W{i}")
            nc.sync.dma_start(wt[:], w_d[k0:k0 + kn, :])
            w_tiles.append(wt)

        # ---- phase 1: per-row-tile params, kappa output, masked phi ----
        # Emitted for all row-tiles before any einsum chunk so the heavy
        # DVE/GpSimd chunk streams are never blocked behind phi latency.
        # Natural order = ascending chunk count (rows sorted by length), so
        # the first phi (smallest lt) unblocks the chunk engines fastest.
        rt_order = list(range(rt_count))
        phims = [None] * rt_count
        for rt in rt_order:
            r0 = rt * P
            nch = n_chunks[rt]
            lt = max(nch * LC, LC)  # phi width actually needed (min 1 chunk)

            # ---- raw = x_aug @ W_aug  (PE transpose then matmul) ----
            x_t = xpool.tile([P, d_aug], FP32, tag="x")
            nc.sync.dma_start(x_t[:], x_d[r0:r0 + P, :])
            xt_ps = psum_t.tile([P, len(kchunks) * P], FP32, tag="xT")
            for i, (k0, kn) in enumerate(kchunks):
                nc.tensor.transpose(xt_ps[0:kn, i * P:i * P + P],
                                    x_t[:, k0:k0 + kn], id_t[:])
            xt_sb = xpool.tile([P, len(kchunks) * P], FP32, tag="xTs")
            for i, (k0, kn) in enumerate(kchunks):
                nc.scalar.copy(xt_sb[0:kn, i * P:i * P + P],
                               xt_ps[0:kn, i * P:i * P + P])
            raw = psum_r.tile([P, 3 * K], FP32, tag="raw")
            for i, (k0, kn) in enumerate(kchunks):
                nc.tensor.matmul(raw[:], xt_sb[0:kn, i * P:i * P + P],
                                 w_tiles[i][:], start=(i == 0),
                                 stop=(i == len(kchunks) - 1))

            # ---- per-row mixture params ----
            # ek/nkap first: they gate the Square stage (phi critical path)
            ek = small.tile([P, K], FP32, tag="ek")
            nc.scalar.activation(ek[:], raw[:, 2 * K:3 * K], AF.Exp)
            pk_t = small.tile([P, K], FP32, tag="pk")
            nc.sync.dma_start(pk_t[:], pk_d[r0:r0 + P, :])
            nkap = small.tile([P, K], FP32, tag="nkap")
            nc.vector.scalar_tensor_tensor(nkap[:], pk_t[:], -1.0, ek[:],
                                           op0=ALU.mult, op1=ALU.subtract)
            ah = small.tile([P, K], FP32, tag="ah")
            nc.scalar.copy(ah[:], raw[:, 0:K])
            nbeta = small.tile([P, K], FP32, tag="nbeta")
            nc.scalar.activation(nbeta[:], raw[:, K:2 * K], AF.Exp)
            clip50 = small.tile([P, K], FP32, tag="clip50")
            nc.scalar.activation(clip50[:], raw[:, K:2 * K], AF.Exp,
                                 bias=ln50_t[:, 0:1], scale=-1.0)
            kap = opool.tile([P, K], FP32, tag="kap")
            nc.vector.tensor_add(kap[:], pk_t[:], ek[:])
            nc.sync.dma_start(kout_d[r0:r0 + P, :], kap[:])
            nc.vector.tensor_scalar_mul(nbeta[:], nbeta[:], -1.0)

            if nch == 0:
                w_acc = opool.tile([P, C], FP32, tag="wacc")
                nc.vector.memset(w_acc[:], 0.0)
                nc.sync.dma_start(wout_d[r0:r0 + P, :], w_acc[:])
                continue

            # ---- phi[l] = sum_k exp(ah_k - min(beta_k d^2, 50)) ----
            # ACT ops batched by function (table swaps are expensive), and
            # per-k sq slices so the ACT->DVE->ACT stages pipeline.
            terms = ppool.tile([P, K, L], FP32, tag="terms")
            sq = ppool.tile([P, K, L], FP32, tag="sq")
            for k in range(K):
                nc.scalar.activation(sq[:, k, 0:lt], u_t[:, 0:lt], AF.Square,
                                     bias=nkap[:, k:k + 1], scale=1.0)
            for k in range(K):
                nc.vector.tensor_scalar_min(sq[:, k, 0:lt], sq[:, k, 0:lt],
                                            clip50[:, k:k + 1])
            for k in range(K):
                nc.scalar.activation(terms[:, k, 0:lt], sq[:, k, 0:lt], AF.Exp,
                                     bias=ah[:, k:k + 1],
                                     scale=nbeta[:, k:k + 1])
            nc.vector.tensor_add(terms[:, 0:5, 0:lt], terms[:, 0:5, 0:lt],
                                 terms[:, 5:10, 0:lt])
            nc.vector.tensor_add(terms[:, 0:2, 0:lt], terms[:, 0:2, 0:lt],
                                 terms[:, 2:4, 0:lt])
            nc.vector.tensor_add(terms[:, 0:1, 0:lt], terms[:, 0:1, 0:lt],
                                 terms[:, 1:2, 0:lt])
            nc.vector.tensor_add(terms[:, 0:1, 0:lt], terms[:, 0:1, 0:lt],
                                 terms[:, 4:5, 0:lt])

            # phi_masked = (u <= seqlen) * phi   (one fused DVE op)
            sl_t = small.tile([P, 1], FP32, tag="sl")
            nc.sync.dma_start(sl_t[:], sl_d[r0:r0 + P, :])
            phim = mpool.tile([P, L], FP32, tag="phim")
            nc.vector.scalar_tensor_tensor(phim[:, 0:lt], u_t[:, 0:lt],
                                           sl_t[:, 0:1], terms[:, 0, 0:lt],
                                           op0=ALU.is_le, op1=ALU.mult)
            phims[rt] = phim

        # ---- phase 2: w = sum_l phi[l] * char[l, :], streamed in l-chunks.
        # Chunks split between DVE and GpSimd (they stream f32 tensor_tensor
        # concurrently: 1x-mode DVE ops don't take the shared SBUF port pair
        # GpSimd uses).
        gci = [0]
        for rt in rt_order:
            r0 = rt * P
            nch = n_chunks[rt]
            phim = phims[rt]
            if phim is None:
                continue
            w_acc = opool.tile([P, C], FP32, tag="wacc")
            assign = [(gci[0] + ci) % GP_MOD in GP_RES for ci in range(nch)]
            gci[0] += nch
            if any(assign):
                w_gp = opool.tile([P, C], FP32, tag="wgp")
            else:
                w_gp = None
            first = {False: True, True: True}  # engine -> first chunk?
            for ci in range(nch):
                use_gp = assign[ci]
                eng = nc.gpsimd if use_gp else nc.vector
                acc = w_gp if use_gp else w_acc
                l0 = ci * LC
                ch_t = chpool.tile([P, LC, C], FP32, tag="ch")
                nc.sync.dma_start(ch_t[:, 0:LC // 2, :],
                                  ch_d[r0:r0 + P, l0:l0 + LC // 2, :])
                nc.sync.dma_start(ch_t[:, LC // 2:LC, :],
                                  ch_d[r0:r0 + P, l0 + LC // 2:l0 + LC, :])
                if TREE_BF16:
                    # multiply writes packed bf16 into the head of the same
                    # buffer (write offset 2i trails read offset 4i, safe);
                    # tree adds then run in DVE 2x_1P mode.
                    prod = ch_t[:].bitcast(mybir.dt.bfloat16)[:, :, 0:C]
                else:
                    prod = ch_t[:]
                eng.tensor_mul(
                    prod[:], ch_t[:],
                    phim[:, l0:l0 + LC].broadcast_to([P, LC, C]))
                h = LC // 2
                while h > 1:
                    eng.tensor_add(prod[:, 0:h, :], prod[:, 0:h, :],
                                   prod[:, h:2 * h, :])
                    h //= 2
                if first[use_gp]:
                    eng.tensor_add(acc[:], prod[:, 0, :], prod[:, 1, :])
                    first[use_gp] = False
                else:
                    eng.tensor_add(prod[:, 0, :], prod[:, 0, :],
                                   prod[:, 1, :])
                    eng.tensor_add(acc[:], prod[:, 0, :], acc[:])
            if not first[True]:  # some chunks went to GpSimd
                if first[False]:
                    nc.vector.tensor_copy(w_acc[:], w_gp[:])
                else:
                    nc.vector.tensor_add(w_acc[:], w_acc[:], w_gp[:])
            nc.sync.dma_start(wout_d[r0:r0 + P, :], w_acc[:])

    nc.compile()
    return nc


def kernel(inputs, prev_kappa, char_seq_one_hot, sequence_lengths, W, b,
           kappa_scale, _profile=False):
    x = np.ascontiguousarray(inputs, dtype=np.float32)
    pk = np.ascontiguousarray(prev_kappa, dtype=np.float32)
    ch = np.ascontiguousarray(char_seq_one_hot, dtype=np.float32)
    sl = np.asarray(sequence_lengths)
    b_ = np.asarray(W, dtype=np.float32), np.asarray(b, dtype=np.float32)
    w_np, b_np = b_
    ks = float(np.asarray(kappa_scale, dtype=np.float32))

    batch = x.shape[0]
    d_in = x.shape[1]
    d_aug = d_in + 1
    assert batch % (P * N_CORES) == 0, f"batch {batch} must be a multiple of 1024"
    rt_count = batch // (P * N_CORES)

    # sort rows by sequence length; slot s of core i <- sorted tile s*8+i
    order = np.argsort(np.asarray(sl, dtype=np.int64), kind="stable")
    core_rows = []
    for i in range(N_CORES):
        core_rows.append(np.concatenate(
            [order[P * (s * N_CORES + i): P * (s * N_CORES + i) + P]
             for s in range(rt_count)]))
    sl_sorted = np.asarray(sl)[order]
    n_chunks = []
    for s in range(rt_count):
        m = int(sl_sorted[P * s * N_CORES: P * (s + 1) * N_CORES].max())
        m = max(0, min(m, L))
        n_chunks.append(-(-m // LC))

    key = (tuple(n_chunks), d_aug, rt_count)
    if key not in _cache:
        _cache[key] = _build(n_chunks, d_aug)
    nc = _cache[key]

    b_adj = b_np.copy()
    b_adj[2 * K:3 * K] += np.float32(ks)
    w_aug = np.concatenate([w_np, b_adj[None, :]], axis=0).astype(np.float32)
    u_np = np.tile(np.arange(1, L + 1, dtype=np.float32)[None, :], (P, 1))
    id_np = np.eye(P, dtype=np.float32)
    sl_f = np.asarray(sl, dtype=np.float32)

    in_maps = []
    for i in range(N_CORES):
        r = core_rows[i]
        xa = np.concatenate([x[r], np.ones((len(r), 1), np.float32)], axis=1)
        in_maps.append({
            "x": np.ascontiguousarray(xa),
            "pk": np.ascontiguousarray(pk[r]),
            "ch": np.ascontiguousarray(ch[r]),
            "sl": np.ascontiguousarray(sl_f[r][:, None]),
            "wmat": w_aug,
            "u": u_np,
            "ident": id_np,
        })

    res = run_bass_kernel_spmd(nc, in_maps, core_ids=list(range(N_CORES)))
    last_run["exec_time_ns"] = res.exec_time_ns
    last_run["profile_json"] = res.profile_json
    last_run["nc"] = nc
    last_run["in_maps"] = in_maps

    w_full = np.empty((batch, C), dtype=np.float32)
    k_full = np.empty((batch, K), dtype=np.float32)
    for i in range(N_CORES):
        w_full[core_rows[i]] = res.results[i]["w_out"]
        k_full[core_rows[i]] = res.results[i]["k_out"]
    return w_full, k_full
